# revision 10
# baseline (speedup 1.0000x reference)
"""Trainium2 Bass kernel for CombinedLossExp72 (feature MSE + triplet + InfoNCE
with hard-negative mining over a 4096x512 codebook).

v2.2 design (data-parallel over batch, 8 cores x 2048 tokens):
  per 128-token tile, 4 column-quarters of the codebook (1024 codes each):
    PE:   W_q = x @ cb^T - c2/2   (fp8 e4m3 DoubleRow, f32r rank-1 bias)
          G_q = x @ cn^T          (fp8 e4m3 DoubleRow)
    DVE:  per-quarter max8 straight from PSUM -> union of 32 candidates ->
          max8/match_replace/max8 -> t = 16th-largest W (union-of-quarters
          is exact unless >8 of the top-16 land in one quarter, ~1% of
          tokens, which only swaps a borderline negative)
    M:    suppression mask from W-PSUM: 2 quarters on ACT (Relu(t-W)) and
          2 on DVE (min(W-t,0)), stored bf16 -> frees W's PSUM banks
    PE:   G' = G -/+ lambda*I @ M  (rank-128 ident matmul into mm2 PSUM)
          so unselected logits become -huge
    ACT:  exp(G' * 1/(T*||x||)) with accum_out == negsum per quarter, free
  The positive code is NOT masked in W; instead its contribution
  (wpos = posdot - p2/2 >= t) * exp(pos logit) is subtracted from negsum.
  p2 and 1/||pos|| ride in from the host (codebook-stat gathers, like the
  positive-vector gather itself).
  Host: shard, bf16/fp8 casts + DoubleRow interleave, codebook stats,
        positive gather, final scalar combine.
"""

import numpy as np
import ml_dtypes
from contextlib import ExitStack

B, T, D, K = 8, 2048, 512, 4096
NCORES = 8
TOK = (B * T) // NCORES      # tokens per core
P = 128
NTILES = TOK // P            # 16
NQ = 4                       # codebook column quarters
QW = K // NQ                 # 1024
NDR = 2                      # DoubleRow contraction chunks (256 rows each)
MARGIN, TEMP = 0.2, 0.1
FEATURE_W, TRIPLET_W, CONTRASTIVE_W = 1.0, 1.0, 0.5
SENT = -2.0e30               # match_replace sentinel
LAMB = 3.0e4                 # suppression slope: exp(-LAMB*gap*rxoT) ~ 0


def emit(tc, ins, outs, ntiles=NTILES):
    import concourse.bass as bass  # noqa: F401
    from concourse import mybir

    nc = tc.nc
    f32 = mybir.dt.float32
    f32r = mybir.dt.float32r
    bf16 = mybir.dt.bfloat16
    f8 = mybir.dt.float8e4
    AF = mybir.ActivationFunctionType
    OP = mybir.AluOpType
    AX = mybir.AxisListType.X
    DR = mybir.MatmulPerfMode.DoubleRow

    x_bf = ins["x_bf"]
    t_bf = ins["t_bf"]
    tn_bf = ins["tn_bf"]
    p_bf = ins["p_bf"]
    xdr = ins["xdr"]
    cbdr = ins["cbdr"]
    cndr = ins["cndr"]
    c2h = ins["c2h_neg"]
    p2h_in = ins["p2h"]
    rp_in = ins["rpos"]
    rxoT_in = ins["rxoT"]
    out_part = outs["out_part"]

    with ExitStack() as ctx:
        const = ctx.enter_context(tc.tile_pool(name="const", bufs=1))
        iop = ctx.enter_context(tc.tile_pool(name="io", bufs=3))
        workW = ctx.enter_context(tc.tile_pool(name="workW", bufs=2))
        sm = ctx.enter_context(tc.tile_pool(name="sm", bufs=6))
        colsp = ctx.enter_context(tc.tile_pool(name="cols", bufs=1))
        scrp = ctx.enter_context(tc.tile_pool(name="scr", bufs=2))
        psum = ctx.enter_context(tc.tile_pool(name="psum", bufs=4, space="PSUM"))

        # ---- constants (loaded once, on the gpsimd DMA queue) ----
        cb_c, cn_c = [], []
        for c in range(NDR):
            cbt = const.tile([P, 2, K], f8, name=f"cb{c}")
            nc.gpsimd.dma_start(cbt[:], cbdr[c])
            cb_c.append(cbt)
        for c in range(NDR):
            cnt_ = const.tile([P, 2, K], f8, name=f"cn{c}")
            nc.gpsimd.dma_start(cnt_[:], cndr[c])
            cn_c.append(cnt_)
        c2h_sb = const.tile([1, K], f32r, name="c2h_sb")
        nc.gpsimd.dma_start(c2h_sb[:], c2h[:])
        ones_sb = const.tile([1, P], f32r, name="ones_sb")
        nc.gpsimd.dma_start(ones_sb[:], ins["ones_in"][:])
        identn_sb = const.tile([P, P], bf16, name="identn_sb")
        nc.gpsimd.dma_start(identn_sb[:], ins["identn_in"][:])
        identp_sb = const.tile([P, P], bf16, name="identp_sb")
        nc.gpsimd.dma_start(identp_sb[:], ins["identp_in"][:])
        margin_sb = const.tile([P, 1], f32, name="margin_sb")
        nc.vector.memset(margin_sb[:], MARGIN)

        fncols = colsp.tile([P, 2 * ntiles], f32, name="fncols")
        tripcols = colsp.tile([P, ntiles], f32, name="tripcols")
        cecols = colsp.tile([P, ntiles], f32, name="cecols")

        pend_mm2 = None   # (t, xdr_t, M, rxoT_t) -> mm2 deferred one tile
        pending = None    # ce finalize deferred until after its mm2/exps
        pend_aux = {}     # t -> (posdot, rxoT_t, rp_t, p2h_t, t16)

        def _emit_mm2(p):
            """mm2 quarters for tile pt: G' = x @ cn^T -/+ LAMB*I @ M_q,
            negsum per quarter from the exp's accum."""
            pt, xdr_p, M_p, rxoT_p = p
            negs4 = sm.tile([P, NQ], f32, tag="negs4")
            for h in range(2):
                pns = {}
                for q in (2 * h, 2 * h + 1):
                    pns[q] = psum.tile([P, QW], f32, tag="psum",
                                       name=f"pn{q}")
                for c in range(NDR):
                    for q in (2 * h, 2 * h + 1):
                        for j in range(QW // 512):
                            js = slice(j * 512, (j + 1) * 512)
                            cs = slice(q * QW + j * 512,
                                       q * QW + (j + 1) * 512)
                            nc.tensor.matmul(pns[q][:, js], xdr_p[:, c, :, :],
                                             cn_c[c][:, :, cs],
                                             start=(c == 0), stop=False,
                                             perf_mode=DR)
                for q in (2 * h, 2 * h + 1):
                    ident = identn_sb if q in (0, 3) else identp_sb
                    for j in range(QW // 512):
                        js = slice(j * 512, (j + 1) * 512)
                        cs = slice(q * QW + j * 512, q * QW + (j + 1) * 512)
                        nc.tensor.matmul(pns[q][:, js], ident[:], M_p[:, cs],
                                         start=False, stop=True)
                    escr = scrp.tile([P, QW], bf16, tag="escr")
                    nc.scalar.activation(escr[:], pns[q][:], AF.Exp,
                                         scale=rxoT_p[:],
                                         accum_out=negs4[:, q:q + 1])
            return negs4

        def _finalize(p):
            pt, posdot, rxoT, rp, p2h, t16, negs4 = p
            negsum = sm.tile([P, 1], f32, tag="negsum")
            nc.vector.tensor_reduce(negsum[:], negs4[:], AX, OP.add)
            l0 = sm.tile([P, 1], f32, tag="l0")
            nc.vector.tensor_scalar(l0[:], posdot[:], rxoT[:], rp[:],
                                    OP.mult, OP.mult)
            posexp = sm.tile([P, 1], f32, tag="posexp")
            nc.scalar.activation(posexp[:], l0[:], AF.Exp)
            # wpos = posdot - p2/2 ; pcorr = (wpos >= t) * posexp
            wpos = sm.tile([P, 1], f32, tag="wpos")
            nc.gpsimd.tensor_tensor(wpos[:], posdot[:], p2h[:], OP.subtract)
            pcorr = sm.tile([P, 1], f32, tag="pcorr")
            nc.vector.scalar_tensor_tensor(pcorr[:], wpos[:], t16, posexp[:],
                                           OP.is_ge, OP.mult)
            u = sm.tile([P, 1], f32, tag="u")
            nc.gpsimd.tensor_tensor(u[:], negsum[:], posexp[:], OP.add)
            u2 = sm.tile([P, 1], f32, tag="u2")
            nc.gpsimd.tensor_tensor(u2[:], u[:], pcorr[:], OP.subtract)
            lse = sm.tile([P, 1], f32, tag="lse")
            nc.scalar.activation(lse[:], u2[:], AF.Ln)
            nc.gpsimd.tensor_tensor(cecols[:, pt:pt + 1], lse[:], l0[:],
                                    OP.subtract)

        for t in range(ntiles):
            rs = slice(t * P, (t + 1) * P)
            x_t = iop.tile([P, D], bf16, tag="x_t")
            nc.sync.dma_start(x_t[:], x_bf[rs, :])
            t_t = iop.tile([P, D], bf16, tag="t_t")
            nc.sync.dma_start(t_t[:], t_bf[rs, :])
            tn_t = iop.tile([P, D], bf16, tag="tn_t")
            nc.sync.dma_start(tn_t[:], tn_bf[rs, :])
            p_t = iop.tile([P, D], bf16, tag="p_t")
            nc.sync.dma_start(p_t[:], p_bf[rs, :])
            xdr_t = iop.tile([P, NDR, 2, P], f8, tag="xdr_t")
            for c in range(NDR):
                nc.sync.dma_start(xdr_t[:, c, :, :], xdr[c, :, :, rs])
            p2h_t = iop.tile([P, 1], f32, tag="p2h_t")
            nc.sync.dma_start(p2h_t[:], p2h_in[rs, :])
            rp_t = iop.tile([P, 1], f32, tag="rp_t")
            nc.sync.dma_start(rp_t[:], rp_in[rs, :])
            rxoT_t = iop.tile([P, 1], f32, tag="rxoT_t")  # 1/(T*||x||), host
            nc.sync.dma_start(rxoT_t[:], rxoT_in[rs, :])

            # ---- positive logit dot (DVE) ----
            sd = scrp.tile([P, D], f32, tag="scr512")
            posdot = sm.tile([P, 1], f32, tag="posdot")
            nc.vector.scalar_tensor_tensor(sd[:], x_t[:], 0.0, p_t[:],
                                           OP.bypass, OP.mult,
                                           accum_out=posdot[:])

            # ---- feature + triplet (POOL subtract, ACT/DVE square + tail) ----
            dsc = scrp.tile([P, D], bf16, tag="dsc")
            nc.gpsimd.tensor_tensor(dsc[:], x_t[:], t_t[:], OP.subtract)
            s2 = scrp.tile([P, D], f32, tag="scr512")
            nc.scalar.activation(s2[:], dsc[:], AF.Square,
                                 accum_out=fncols[:, 2 * t:2 * t + 1])
            nsc = scrp.tile([P, D], bf16, tag="dsc")
            nc.gpsimd.tensor_tensor(nsc[:], x_t[:], tn_t[:], OP.subtract)
            s3 = scrp.tile([P, D], f32, tag="scr512")
            nc.vector.scalar_tensor_tensor(s3[:], nsc[:], 0.0, nsc[:],
                                           OP.bypass, OP.mult,
                                           accum_out=fncols[:, 2 * t + 1:2 * t + 2])
            ld2 = sm.tile([P, 2], f32, tag="ld2")
            nc.scalar.activation(ld2[:], fncols[:, 2 * t:2 * t + 2], AF.Ln)
            pn2 = sm.tile([P, 2], f32, tag="pn2")         # [pos_dist, neg_dist]
            nc.scalar.activation(pn2[:], ld2[:], AF.Exp, scale=0.5)
            tv = sm.tile([P, 1], f32, tag="tv")
            nc.gpsimd.tensor_tensor(tv[:], pn2[:, 0:1], pn2[:, 1:2],
                                    OP.subtract)
            nc.scalar.activation(tripcols[:, t:t + 1], tv[:], AF.Relu,
                                 bias=margin_sb[:])

            Wsb = workW.tile([P, K], f32, tag="Wsb")
            M = workW.tile([P, K], bf16, tag="M")
            mall = sm.tile([P, NQ * 8], f32, tag="mall")

            # ---- mm1: W = x @ cb^T - c2/2 (fp8 DR + f32r bias);
            # bias MMs hoisted (one ones-LDWEIGHTS), DR in quarter pairs
            # with c-consecutive ordering so weights reload 2x per pair;
            # DVE max8 from PSUM; W spilled to SBUF to recycle banks
            pqs = [psum.tile([P, QW], f32, tag="psum", name=f"pg{q}")
                   for q in range(NQ)]
            for q in range(NQ):
                for j in range(QW // 512):
                    js = slice(j * 512, (j + 1) * 512)
                    cs = slice(q * QW + j * 512, q * QW + (j + 1) * 512)
                    nc.tensor.matmul(pqs[q][:, js], ones_sb[:], c2h_sb[:, cs],
                                     start=True, stop=False)
            for h in range(2):
                for c in range(NDR):
                    last = c == NDR - 1
                    for q in (2 * h, 2 * h + 1):
                        for j in range(QW // 512):
                            js = slice(j * 512, (j + 1) * 512)
                            cs = slice(q * QW + j * 512,
                                       q * QW + (j + 1) * 512)
                            nc.tensor.matmul(pqs[q][:, js], xdr_t[:, c, :, :],
                                             cb_c[c][:, :, cs], start=False,
                                             stop=last, perf_mode=DR)
                for q in (2 * h, 2 * h + 1):
                    qs = slice(q * QW, (q + 1) * QW)
                    nc.vector.max(mall[:, q * 8:(q + 1) * 8], pqs[q][:])
                    if q < 2:
                        nc.scalar.activation(Wsb[:, qs], pqs[q][:], AF.Copy)
                    else:
                        nc.vector.tensor_copy(Wsb[:, qs], pqs[q][:])

            # mm2 of the PREVIOUS tile FIRST: its mask is ready, and its
            # exps must not queue behind this tile's t16-gated masks on ACT
            # (the exps release the PSUM banks the next mm1 needs).
            if pend_mm2 is not None:
                negs4 = _emit_mm2(pend_mm2)
                pt = pend_mm2[0]
                if pending is not None:
                    _finalize(pending)
                pending = (pt, *pend_aux[pt], negs4)

            # ---- selection merge: t16 = 16th largest over candidates ----
            c1 = sm.tile([P, 8], f32, tag="c1")
            nc.vector.max(c1[:], mall[:])
            mrep = sm.tile([P, NQ * 8], f32, tag="mrep")
            nc.vector.match_replace(mrep[:], c1[:], mall[:], SENT)
            c2t = sm.tile([P, 8], f32, tag="c2t")
            nc.vector.max(c2t[:], mrep[:])
            t16 = c2t[:, 7:8]

            # ---- suppression mask M from W-SBUF (2 quarters ACT, 2 DVE) ----
            # q0/q3 (ACT):  M = relu(t - W)      >= 0  -> paired with -LAMB*I
            # q1/q2 (DVE):  M = min(W - t, 0)    <= 0  -> paired with +LAMB*I
            for q in range(NQ):
                qs = slice(q * QW, (q + 1) * QW)
                if q in (0, 3):
                    nc.scalar.activation(M[:, qs], Wsb[:, qs], AF.Relu,
                                         scale=-1.0, bias=t16)
                else:
                    nc.vector.tensor_scalar(M[:, qs], Wsb[:, qs], t16, 0.0,
                                            OP.subtract, OP.min)

            pend_mm2 = (t, xdr_t, M, rxoT_t)
            pend_aux[t] = (posdot, rxoT_t, rp_t, p2h_t, t16)

        # epilogue: last tile's mm2, then the last two finalizations
        negs4 = _emit_mm2(pend_mm2)
        pt = pend_mm2[0]
        if pending is not None:
            _finalize(pending)
        _finalize((pt, *pend_aux[pt], negs4))

        outsb = colsp.tile([P, 4], f32, name="outsb")
        nc.vector.memset(outsb[:, 3:4], 0.0)
        nc.vector.tensor_reduce(outsb[:, 0:1], fncols[:, 0:2 * ntiles:2],
                                AX, OP.add)
        nc.vector.tensor_reduce(outsb[:, 1:2], tripcols[:], AX, OP.add)
        nc.vector.tensor_reduce(outsb[:, 2:3], cecols[:], AX, OP.add)
        nc.sync.dma_start(out_part[:], outsb[:])


def _patch_act_tables():
    """Bias the act-table-load placement pass toward the one set
    (natural_log_exp_and_others) that contains every func this kernel uses
    (ln/exp/relu/copy), so the whole program needs a single table load."""
    import concourse.bacc as bacc_mod
    if getattr(bacc_mod, "_act_tables_patched", False):
        return
    orig = bacc_mod.get_activation_tables
    target = "natural_log_exp_and_others"

    def patched(module_arch):
        tabs = orig(module_arch)
        full = tabs[target]
        return {name: (s if name == target else s - full)
                for name, s in tabs.items()}

    bacc_mod.get_activation_tables = patched
    bacc_mod._act_tables_patched = True


def build(ntiles=NTILES, nreps=1):
    """Build + compile the Bacc program. Returns nc.

    nreps>1 repeats the full body inside one NEFF (used by test.py to
    amortize dispatch overhead out of the HW timing measurement)."""
    import concourse.bacc as bacc
    import concourse.tile as tile
    from concourse import mybir

    _patch_act_tables()

    f32 = mybir.dt.float32
    f32r = mybir.dt.float32r
    bf16 = mybir.dt.bfloat16
    f8 = mybir.dt.float8e4

    nc = bacc.Bacc("TRN2", target_bir_lowering=False, debug=False,
                   enable_asserts=False, num_devices=NCORES)
    ins = {
        "x_bf": nc.dram_tensor("x_bf", [TOK, D], bf16, kind="ExternalInput").ap(),
        "t_bf": nc.dram_tensor("t_bf", [TOK, D], bf16, kind="ExternalInput").ap(),
        "tn_bf": nc.dram_tensor("tn_bf", [TOK, D], bf16, kind="ExternalInput").ap(),
        "p_bf": nc.dram_tensor("p_bf", [TOK, D], bf16, kind="ExternalInput").ap(),
        "xdr": nc.dram_tensor("xdr", [NDR, P, 2, TOK], f8, kind="ExternalInput").ap(),
        "cbdr": nc.dram_tensor("cbdr", [NDR, P, 2, K], f8, kind="ExternalInput").ap(),
        "cndr": nc.dram_tensor("cndr", [NDR, P, 2, K], f8, kind="ExternalInput").ap(),
        "c2h_neg": nc.dram_tensor("c2h_neg", [1, K], f32r, kind="ExternalInput").ap(),
        "ones_in": nc.dram_tensor("ones_in", [1, P], f32r, kind="ExternalInput").ap(),
        "identn_in": nc.dram_tensor("identn_in", [P, P], bf16, kind="ExternalInput").ap(),
        "identp_in": nc.dram_tensor("identp_in", [P, P], bf16, kind="ExternalInput").ap(),
        "p2h": nc.dram_tensor("p2h", [TOK, 1], f32, kind="ExternalInput").ap(),
        "rpos": nc.dram_tensor("rpos", [TOK, 1], f32, kind="ExternalInput").ap(),
        "rxoT": nc.dram_tensor("rxoT", [TOK, 1], f32, kind="ExternalInput").ap(),
    }
    outs = {
        "out_part": nc.dram_tensor("out_part", [P, 4], f32, kind="ExternalOutput").ap(),
    }
    with tile.TileContext(nc) as tc:
        for _rep in range(nreps):
            emit(tc, ins, outs, ntiles=ntiles)
    nc.compile()
    return nc


def _dr_interleave(a_dxn):
    """[D, N] -> DoubleRow lhsT layout [NDR, 128, 2, N] (fp8)."""
    Dn, N = a_dxn.shape
    assert Dn == D
    out = a_dxn.reshape(NDR, 2, P, N).transpose(0, 2, 1, 3)
    return np.ascontiguousarray(out).astype(ml_dtypes.float8_e4m3fn)


def make_in_maps(student_features, teacher_features, codebook, teacher_codes):
    """Host-side shard + layout prep. Returns list of 8 per-core input dicts."""
    bf = ml_dtypes.bfloat16
    x = np.asarray(student_features, dtype=np.float32).reshape(B * T, D)
    tch = np.asarray(teacher_features, dtype=np.float32).reshape(B, T, D)
    cb = np.asarray(codebook, dtype=np.float32)
    codes = np.asarray(teacher_codes).reshape(B * T).astype(np.int64)

    c2 = (cb.astype(np.float64) ** 2).sum(axis=1)
    cn = (cb / np.sqrt(c2)[:, None]).astype(np.float32)

    cbdr = _dr_interleave(np.ascontiguousarray(cb.T))
    cndr = _dr_interleave(np.ascontiguousarray(cn.T))
    c2h_neg = (-0.5 * c2).astype(np.float32).reshape(1, K)
    ones = np.ones((1, P), dtype=np.float32)
    identn = (np.eye(P, dtype=np.float32) * -LAMB).astype(bf)
    identp = (np.eye(P, dtype=np.float32) * LAMB).astype(bf)
    p2h_all = (0.5 * c2[codes]).astype(np.float32).reshape(B * T, 1)
    rp_all = (1.0 / np.sqrt(c2[codes])).astype(np.float32).reshape(B * T, 1)
    x2_all = np.einsum("nd,nd->n", x, x, dtype=np.float64)
    rxoT_all = (1.0 / (TEMP * np.sqrt(x2_all))).astype(np.float32).reshape(
        B * T, 1)

    in_maps = []
    for b in range(NCORES):
        sl = slice(b * TOK, (b + 1) * TOK)
        xs = x[sl]
        codes_s = codes[sl]
        in_maps.append({
            "x_bf": xs.astype(bf),
            "t_bf": tch[b].astype(bf),
            "tn_bf": tch[(b - 1) % B].astype(bf),
            "p_bf": cb[codes_s].astype(bf),
            "xdr": _dr_interleave(np.ascontiguousarray(xs.T)),
            "cbdr": cbdr,
            "cndr": cndr,
            "c2h_neg": c2h_neg,
            "ones_in": ones,
            "identn_in": identn,
            "identp_in": identp,
            "p2h": p2h_all[sl],
            "rpos": rp_all[sl],
            "rxoT": rxoT_all[sl],
        })
    return in_maps


def combine(results):
    """Combine per-core [128, 4] partials into the scalar loss."""
    feat = trip = ce = 0.0
    for r in results:
        p = np.asarray(r["out_part"], dtype=np.float64)
        feat += p[:, 0].sum()
        trip += p[:, 1].sum()
        ce += p[:, 2].sum()
    n = float(B * T)
    total = (FEATURE_W * feat / (n * D)
             + TRIPLET_W * trip / n
             + CONTRASTIVE_W * ce / n)
    return np.float32(total)


_NC_CACHE = None


def kernel(student_features, teacher_features, codebook, teacher_codes):
    global _NC_CACHE
    from concourse import bass_utils

    if _NC_CACHE is None:
        _NC_CACHE = build()
    nc = _NC_CACHE
    in_maps = make_in_maps(student_features, teacher_features, codebook,
                           teacher_codes)
    res = bass_utils.run_bass_kernel_spmd(nc, in_maps,
                                          core_ids=list(range(NCORES)))
    return combine(res.results)



# revision 11
# speedup vs baseline: 1.0274x; 1.0274x over previous
"""Trainium2 Bass kernel for CombinedLossExp72 (feature MSE + triplet + InfoNCE
with hard-negative mining over a 4096x512 codebook).

v2.2 design (data-parallel over batch, 8 cores x 2048 tokens):
  per 128-token tile, 4 column-quarters of the codebook (1024 codes each):
    PE:   W_q = x @ cb^T - c2/2   (fp8 e4m3 DoubleRow, f32r rank-1 bias)
          G_q = x @ cn^T          (fp8 e4m3 DoubleRow)
    DVE:  per-quarter max8 straight from PSUM -> union of 32 candidates ->
          max8/match_replace/max8 -> t = 16th-largest W (union-of-quarters
          is exact unless >8 of the top-16 land in one quarter, ~1% of
          tokens, which only swaps a borderline negative)
    M:    suppression mask from W-PSUM: 2 quarters on ACT (Relu(t-W)) and
          2 on DVE (min(W-t,0)), stored bf16 -> frees W's PSUM banks
    PE:   G' = G -/+ lambda*I @ M  (rank-128 ident matmul into mm2 PSUM)
          so unselected logits become -huge
    ACT:  exp(G' * 1/(T*||x||)) with accum_out == negsum per quarter, free
  The positive code is NOT masked in W; instead its contribution
  (wpos = posdot - p2/2 >= t) * exp(pos logit) is subtracted from negsum.
  p2 and 1/||pos|| ride in from the host (codebook-stat gathers, like the
  positive-vector gather itself).
  Host: shard, bf16/fp8 casts + DoubleRow interleave, codebook stats,
        positive gather, final scalar combine.
"""

import numpy as np
import ml_dtypes
from contextlib import ExitStack

B, T, D, K = 8, 2048, 512, 4096
NCORES = 8
TOK = (B * T) // NCORES      # tokens per core
P = 128
NTILES = TOK // P            # 16
NQ = 4                       # codebook column quarters
QW = K // NQ                 # 1024
NDR = 2                      # DoubleRow contraction chunks (256 rows each)
MARGIN, TEMP = 0.2, 0.1
FEATURE_W, TRIPLET_W, CONTRASTIVE_W = 1.0, 1.0, 0.5
SENT = -2.0e30               # match_replace sentinel
LAMB = 3.0e4                 # suppression slope: exp(-LAMB*gap*rxoT) ~ 0


def emit(tc, ins, outs, ntiles=NTILES):
    import concourse.bass as bass  # noqa: F401
    from concourse import mybir

    nc = tc.nc
    f32 = mybir.dt.float32
    f32r = mybir.dt.float32r
    bf16 = mybir.dt.bfloat16
    f8 = mybir.dt.float8e4
    AF = mybir.ActivationFunctionType
    OP = mybir.AluOpType
    AX = mybir.AxisListType.X
    DR = mybir.MatmulPerfMode.DoubleRow

    x_bf = ins["x_bf"]
    t_bf = ins["t_bf"]
    tn_bf = ins["tn_bf"]
    p_bf = ins["p_bf"]
    xdr = ins["xdr"]
    cbdr = ins["cbdr"]
    cndr = ins["cndr"]
    c2h = ins["c2h_neg"]
    p2h_in = ins["p2h"]
    rp_in = ins["rpos"]
    rxoT_in = ins["rxoT"]
    out_part = outs["out_part"]

    with ExitStack() as ctx:
        const = ctx.enter_context(tc.tile_pool(name="const", bufs=1))
        iop = ctx.enter_context(tc.tile_pool(name="io", bufs=3))
        workW = ctx.enter_context(tc.tile_pool(name="workW", bufs=2))
        sm = ctx.enter_context(tc.tile_pool(name="sm", bufs=6))
        colsp = ctx.enter_context(tc.tile_pool(name="cols", bufs=1))
        scrp = ctx.enter_context(tc.tile_pool(name="scr", bufs=2))
        psum = ctx.enter_context(tc.tile_pool(name="psum", bufs=4, space="PSUM"))

        # ---- constants (loaded once, on the gpsimd DMA queue) ----
        cb_c, cn_c = [], []
        for c in range(NDR):
            cbt = const.tile([P, 2, K], f8, name=f"cb{c}")
            nc.gpsimd.dma_start(cbt[:], cbdr[c])
            cb_c.append(cbt)
        for c in range(NDR):
            cnt_ = const.tile([P, 2, K], f8, name=f"cn{c}")
            nc.gpsimd.dma_start(cnt_[:], cndr[c])
            cn_c.append(cnt_)
        c2h_sb = const.tile([1, K], f32r, name="c2h_sb")
        nc.gpsimd.dma_start(c2h_sb[:], c2h[:])
        ones_sb = const.tile([1, P], f32r, name="ones_sb")
        nc.gpsimd.dma_start(ones_sb[:], ins["ones_in"][:])
        identn_sb = const.tile([P, P], bf16, name="identn_sb")
        nc.gpsimd.dma_start(identn_sb[:], ins["identn_in"][:])
        identp_sb = const.tile([P, P], bf16, name="identp_sb")
        nc.gpsimd.dma_start(identp_sb[:], ins["identp_in"][:])
        margin_sb = const.tile([P, 1], f32, name="margin_sb")
        nc.vector.memset(margin_sb[:], MARGIN)

        fncols = colsp.tile([P, 2 * ntiles], f32, name="fncols")
        tripcols = colsp.tile([P, ntiles], f32, name="tripcols")
        cecols = colsp.tile([P, ntiles], f32, name="cecols")

        pend_mm2 = None   # (t, xdr_t, M, rxoT_t) -> mm2 deferred one tile
        pending = None    # ce finalize deferred until after its mm2/exps
        pend_aux = {}     # t -> (posdot, rxoT_t, rp_t, p2h_t, t16)

        def _emit_mm2(p):
            """mm2 quarters for tile pt: G' = x @ cn^T -/+ LAMB*I @ M_q,
            negsum per quarter from the exp's accum."""
            pt, xdr_p, M_p, rxoT_p = p
            negs4 = sm.tile([P, NQ], f32, tag="negs4")
            for h in range(2):
                pns = {}
                for q in (2 * h, 2 * h + 1):
                    pns[q] = psum.tile([P, QW], f32, tag="psum",
                                       name=f"pn{q}")
                for c in range(NDR):
                    for q in (2 * h, 2 * h + 1):
                        for j in range(QW // 512):
                            js = slice(j * 512, (j + 1) * 512)
                            cs = slice(q * QW + j * 512,
                                       q * QW + (j + 1) * 512)
                            nc.tensor.matmul(pns[q][:, js], xdr_p[:, c, :, :],
                                             cn_c[c][:, :, cs],
                                             start=(c == 0), stop=False,
                                             perf_mode=DR)
                for q in (2 * h, 2 * h + 1):
                    ident = identn_sb if q in (0, 3) else identp_sb
                    for j in range(QW // 512):
                        js = slice(j * 512, (j + 1) * 512)
                        cs = slice(q * QW + j * 512, q * QW + (j + 1) * 512)
                        nc.tensor.matmul(pns[q][:, js], ident[:], M_p[:, cs],
                                         start=False, stop=True)
                    escr = scrp.tile([P, QW], bf16, tag="escr")
                    nc.scalar.activation(escr[:], pns[q][:], AF.Exp,
                                         scale=rxoT_p[:],
                                         accum_out=negs4[:, q:q + 1])
            return negs4

        def _finalize(p):
            pt, posdot, rxoT, rp, p2h, t16, negs4 = p
            negsum = sm.tile([P, 1], f32, tag="negsum")
            nc.vector.tensor_reduce(negsum[:], negs4[:], AX, OP.add)
            l0 = sm.tile([P, 1], f32, tag="l0")
            nc.vector.tensor_scalar(l0[:], posdot[:], rxoT[:], rp[:],
                                    OP.mult, OP.mult)
            posexp = sm.tile([P, 1], f32, tag="posexp")
            nc.scalar.activation(posexp[:], l0[:], AF.Exp)
            # wpos = posdot - p2/2 ; pcorr = (wpos >= t) * posexp
            wpos = sm.tile([P, 1], f32, tag="wpos")
            nc.gpsimd.tensor_tensor(wpos[:], posdot[:], p2h[:], OP.subtract)
            pcorr = sm.tile([P, 1], f32, tag="pcorr")
            nc.vector.scalar_tensor_tensor(pcorr[:], wpos[:], t16, posexp[:],
                                           OP.is_ge, OP.mult)
            u = sm.tile([P, 1], f32, tag="u")
            nc.gpsimd.tensor_tensor(u[:], negsum[:], posexp[:], OP.add)
            u2 = sm.tile([P, 1], f32, tag="u2")
            nc.gpsimd.tensor_tensor(u2[:], u[:], pcorr[:], OP.subtract)
            lse = sm.tile([P, 1], f32, tag="lse")
            nc.scalar.activation(lse[:], u2[:], AF.Ln)
            nc.gpsimd.tensor_tensor(cecols[:, pt:pt + 1], lse[:], l0[:],
                                    OP.subtract)

        for t in range(ntiles):
            rs = slice(t * P, (t + 1) * P)
            x_t = iop.tile([P, D], bf16, tag="x_t")
            nc.sync.dma_start(x_t[:], x_bf[rs, :])
            t_t = iop.tile([P, D], bf16, tag="t_t")
            nc.sync.dma_start(t_t[:], t_bf[rs, :])
            tn_t = iop.tile([P, D], bf16, tag="tn_t")
            nc.sync.dma_start(tn_t[:], tn_bf[rs, :])
            p_t = iop.tile([P, D], bf16, tag="p_t")
            nc.sync.dma_start(p_t[:], p_bf[rs, :])
            xdr_t = iop.tile([P, NDR, 2, P], f8, tag="xdr_t")
            for c in range(NDR):
                nc.sync.dma_start(xdr_t[:, c, :, :], xdr[c, :, :, rs])
            p2h_t = iop.tile([P, 1], f32, tag="p2h_t")
            nc.sync.dma_start(p2h_t[:], p2h_in[rs, :])
            rp_t = iop.tile([P, 1], f32, tag="rp_t")
            nc.sync.dma_start(rp_t[:], rp_in[rs, :])
            rxoT_t = iop.tile([P, 1], f32, tag="rxoT_t")  # 1/(T*||x||), host
            nc.sync.dma_start(rxoT_t[:], rxoT_in[rs, :])

            # ---- positive logit dot (DVE) ----
            sd = scrp.tile([P, D], f32, tag="scr512")
            posdot = sm.tile([P, 1], f32, tag="posdot")
            nc.vector.scalar_tensor_tensor(sd[:], x_t[:], 0.0, p_t[:],
                                           OP.bypass, OP.mult,
                                           accum_out=posdot[:])

            # ---- feature + triplet (POOL subtract, ACT/DVE square + tail) ----
            dsc = scrp.tile([P, D], bf16, tag="dsc")
            nc.gpsimd.tensor_tensor(dsc[:], x_t[:], t_t[:], OP.subtract)
            s2 = scrp.tile([P, D], f32, tag="scr512")
            nc.scalar.activation(s2[:], dsc[:], AF.Square,
                                 accum_out=fncols[:, 2 * t:2 * t + 1])
            nsc = scrp.tile([P, D], bf16, tag="dsc")
            nc.gpsimd.tensor_tensor(nsc[:], x_t[:], tn_t[:], OP.subtract)
            s3 = scrp.tile([P, D], f32, tag="scr512")
            nc.vector.scalar_tensor_tensor(s3[:], nsc[:], 0.0, nsc[:],
                                           OP.bypass, OP.mult,
                                           accum_out=fncols[:, 2 * t + 1:2 * t + 2])
            ld2 = sm.tile([P, 2], f32, tag="ld2")
            nc.scalar.activation(ld2[:], fncols[:, 2 * t:2 * t + 2], AF.Ln)
            pn2 = sm.tile([P, 2], f32, tag="pn2")         # [pos_dist, neg_dist]
            nc.scalar.activation(pn2[:], ld2[:], AF.Exp, scale=0.5)
            tv = sm.tile([P, 1], f32, tag="tv")
            nc.gpsimd.tensor_tensor(tv[:], pn2[:, 0:1], pn2[:, 1:2],
                                    OP.subtract)
            nc.scalar.activation(tripcols[:, t:t + 1], tv[:], AF.Relu,
                                 bias=margin_sb[:])

            Wsb = workW.tile([P, K], f32, tag="Wsb")
            M = workW.tile([P, K], bf16, tag="M")
            mall = sm.tile([P, NQ * 8], f32, tag="mall")

            # ---- mm1: W = x @ cb^T - c2/2 (fp8 DR + f32r bias);
            # bias MMs hoisted (one ones-LDWEIGHTS), DR in quarter pairs
            # with c-consecutive ordering so weights reload 2x per pair;
            # DVE max8 from PSUM; W spilled to SBUF to recycle banks
            pqs = [psum.tile([P, QW], f32, tag="psum", name=f"pg{q}")
                   for q in range(NQ)]
            for q in range(NQ):
                for j in range(QW // 512):
                    js = slice(j * 512, (j + 1) * 512)
                    cs = slice(q * QW + j * 512, q * QW + (j + 1) * 512)
                    nc.tensor.matmul(pqs[q][:, js], ones_sb[:], c2h_sb[:, cs],
                                     start=True, stop=False)
            for h in range(2):
                for c in range(NDR):
                    last = c == NDR - 1
                    for q in (2 * h, 2 * h + 1):
                        for j in range(QW // 512):
                            js = slice(j * 512, (j + 1) * 512)
                            cs = slice(q * QW + j * 512,
                                       q * QW + (j + 1) * 512)
                            nc.tensor.matmul(pqs[q][:, js], xdr_t[:, c, :, :],
                                             cb_c[c][:, :, cs], start=False,
                                             stop=last, perf_mode=DR)
                for q in (2 * h, 2 * h + 1):
                    qs = slice(q * QW, (q + 1) * QW)
                    nc.vector.max(mall[:, q * 8:(q + 1) * 8], pqs[q][:])
                    if q < 2:
                        nc.scalar.activation(Wsb[:, qs], pqs[q][:], AF.Copy)
                    else:
                        nc.vector.tensor_copy(Wsb[:, qs], pqs[q][:])

            # ---- selection merge: t16 = 16th largest over candidates ----
            c1 = sm.tile([P, 8], f32, tag="c1")
            nc.vector.max(c1[:], mall[:])
            mrep = sm.tile([P, NQ * 8], f32, tag="mrep")
            nc.vector.match_replace(mrep[:], c1[:], mall[:], SENT)
            c2t = sm.tile([P, 8], f32, tag="c2t")
            nc.vector.max(c2t[:], mrep[:])
            t16 = c2t[:, 7:8]

            # ---- suppression mask M from W-SBUF (2 quarters ACT, 2 DVE) ----
            # q0/q3 (ACT):  M = relu(t - W)      >= 0  -> paired with -LAMB*I
            # q1/q2 (DVE):  M = min(W - t, 0)    <= 0  -> paired with +LAMB*I
            for q in range(NQ):
                qs = slice(q * QW, (q + 1) * QW)
                if q in (0, 3):
                    nc.scalar.activation(M[:, qs], Wsb[:, qs], AF.Relu,
                                         scale=-1.0, bias=t16)
                else:
                    nc.vector.tensor_scalar(M[:, qs], Wsb[:, qs], t16, 0.0,
                                            OP.subtract, OP.min)

            # mm2 of the PREVIOUS tile (its mask M is ready, so its PSUM
            # groups open and close without straddling the t16 barrier),
            # then ce finalization one tile further back.
            if pend_mm2 is not None:
                negs4 = _emit_mm2(pend_mm2)
                pt = pend_mm2[0]
                if pending is not None:
                    _finalize(pending)
                pending = (pt, *pend_aux[pt], negs4)
            pend_mm2 = (t, xdr_t, M, rxoT_t)
            pend_aux[t] = (posdot, rxoT_t, rp_t, p2h_t, t16)

        # epilogue: last tile's mm2, then the last two finalizations
        negs4 = _emit_mm2(pend_mm2)
        pt = pend_mm2[0]
        if pending is not None:
            _finalize(pending)
        _finalize((pt, *pend_aux[pt], negs4))

        outsb = colsp.tile([P, 4], f32, name="outsb")
        nc.vector.memset(outsb[:, 3:4], 0.0)
        nc.vector.tensor_reduce(outsb[:, 0:1], fncols[:, 0:2 * ntiles:2],
                                AX, OP.add)
        nc.vector.tensor_reduce(outsb[:, 1:2], tripcols[:], AX, OP.add)
        nc.vector.tensor_reduce(outsb[:, 2:3], cecols[:], AX, OP.add)
        nc.sync.dma_start(out_part[:], outsb[:])


def _patch_act_tables():
    """Bias the act-table-load placement pass toward the one set
    (natural_log_exp_and_others) that contains every func this kernel uses
    (ln/exp/relu/copy), so the whole program needs a single table load."""
    import concourse.bacc as bacc_mod
    if getattr(bacc_mod, "_act_tables_patched", False):
        return
    orig = bacc_mod.get_activation_tables
    target = "natural_log_exp_and_others"

    def patched(module_arch):
        tabs = orig(module_arch)
        full = tabs[target]
        return {name: (s if name == target else s - full)
                for name, s in tabs.items()}

    bacc_mod.get_activation_tables = patched
    bacc_mod._act_tables_patched = True


def build(ntiles=NTILES, nreps=1):
    """Build + compile the Bacc program. Returns nc.

    nreps>1 repeats the full body inside one NEFF (used by test.py to
    amortize dispatch overhead out of the HW timing measurement)."""
    import concourse.bacc as bacc
    import concourse.tile as tile
    from concourse import mybir

    _patch_act_tables()

    f32 = mybir.dt.float32
    f32r = mybir.dt.float32r
    bf16 = mybir.dt.bfloat16
    f8 = mybir.dt.float8e4

    nc = bacc.Bacc("TRN2", target_bir_lowering=False, debug=False,
                   enable_asserts=False, num_devices=NCORES)
    ins = {
        "x_bf": nc.dram_tensor("x_bf", [TOK, D], bf16, kind="ExternalInput").ap(),
        "t_bf": nc.dram_tensor("t_bf", [TOK, D], bf16, kind="ExternalInput").ap(),
        "tn_bf": nc.dram_tensor("tn_bf", [TOK, D], bf16, kind="ExternalInput").ap(),
        "p_bf": nc.dram_tensor("p_bf", [TOK, D], bf16, kind="ExternalInput").ap(),
        "xdr": nc.dram_tensor("xdr", [NDR, P, 2, TOK], f8, kind="ExternalInput").ap(),
        "cbdr": nc.dram_tensor("cbdr", [NDR, P, 2, K], f8, kind="ExternalInput").ap(),
        "cndr": nc.dram_tensor("cndr", [NDR, P, 2, K], f8, kind="ExternalInput").ap(),
        "c2h_neg": nc.dram_tensor("c2h_neg", [1, K], f32r, kind="ExternalInput").ap(),
        "ones_in": nc.dram_tensor("ones_in", [1, P], f32r, kind="ExternalInput").ap(),
        "identn_in": nc.dram_tensor("identn_in", [P, P], bf16, kind="ExternalInput").ap(),
        "identp_in": nc.dram_tensor("identp_in", [P, P], bf16, kind="ExternalInput").ap(),
        "p2h": nc.dram_tensor("p2h", [TOK, 1], f32, kind="ExternalInput").ap(),
        "rpos": nc.dram_tensor("rpos", [TOK, 1], f32, kind="ExternalInput").ap(),
        "rxoT": nc.dram_tensor("rxoT", [TOK, 1], f32, kind="ExternalInput").ap(),
    }
    outs = {
        "out_part": nc.dram_tensor("out_part", [P, 4], f32, kind="ExternalOutput").ap(),
    }
    with tile.TileContext(nc) as tc:
        for _rep in range(nreps):
            emit(tc, ins, outs, ntiles=ntiles)
    nc.compile()
    return nc


def _dr_interleave(a_dxn):
    """[D, N] -> DoubleRow lhsT layout [NDR, 128, 2, N] (fp8)."""
    Dn, N = a_dxn.shape
    assert Dn == D
    out = a_dxn.reshape(NDR, 2, P, N).transpose(0, 2, 1, 3)
    return np.ascontiguousarray(out).astype(ml_dtypes.float8_e4m3fn)


def make_in_maps(student_features, teacher_features, codebook, teacher_codes):
    """Host-side shard + layout prep. Returns list of 8 per-core input dicts."""
    bf = ml_dtypes.bfloat16
    x = np.asarray(student_features, dtype=np.float32).reshape(B * T, D)
    tch = np.asarray(teacher_features, dtype=np.float32).reshape(B, T, D)
    cb = np.asarray(codebook, dtype=np.float32)
    codes = np.asarray(teacher_codes).reshape(B * T).astype(np.int64)

    c2 = (cb.astype(np.float64) ** 2).sum(axis=1)
    cn = (cb / np.sqrt(c2)[:, None]).astype(np.float32)

    cbdr = _dr_interleave(np.ascontiguousarray(cb.T))
    cndr = _dr_interleave(np.ascontiguousarray(cn.T))
    c2h_neg = (-0.5 * c2).astype(np.float32).reshape(1, K)
    ones = np.ones((1, P), dtype=np.float32)
    identn = (np.eye(P, dtype=np.float32) * -LAMB).astype(bf)
    identp = (np.eye(P, dtype=np.float32) * LAMB).astype(bf)
    p2h_all = (0.5 * c2[codes]).astype(np.float32).reshape(B * T, 1)
    rp_all = (1.0 / np.sqrt(c2[codes])).astype(np.float32).reshape(B * T, 1)
    x2_all = np.einsum("nd,nd->n", x, x, dtype=np.float64)
    rxoT_all = (1.0 / (TEMP * np.sqrt(x2_all))).astype(np.float32).reshape(
        B * T, 1)

    in_maps = []
    for b in range(NCORES):
        sl = slice(b * TOK, (b + 1) * TOK)
        xs = x[sl]
        codes_s = codes[sl]
        in_maps.append({
            "x_bf": xs.astype(bf),
            "t_bf": tch[b].astype(bf),
            "tn_bf": tch[(b - 1) % B].astype(bf),
            "p_bf": cb[codes_s].astype(bf),
            "xdr": _dr_interleave(np.ascontiguousarray(xs.T)),
            "cbdr": cbdr,
            "cndr": cndr,
            "c2h_neg": c2h_neg,
            "ones_in": ones,
            "identn_in": identn,
            "identp_in": identp,
            "p2h": p2h_all[sl],
            "rpos": rp_all[sl],
            "rxoT": rxoT_all[sl],
        })
    return in_maps


def combine(results):
    """Combine per-core [128, 4] partials into the scalar loss."""
    feat = trip = ce = 0.0
    for r in results:
        p = np.asarray(r["out_part"], dtype=np.float64)
        feat += p[:, 0].sum()
        trip += p[:, 1].sum()
        ce += p[:, 2].sum()
    n = float(B * T)
    total = (FEATURE_W * feat / (n * D)
             + TRIPLET_W * trip / n
             + CONTRASTIVE_W * ce / n)
    return np.float32(total)


_NC_CACHE = None


def kernel(student_features, teacher_features, codebook, teacher_codes):
    global _NC_CACHE
    from concourse import bass_utils

    if _NC_CACHE is None:
        _NC_CACHE = build()
    nc = _NC_CACHE
    in_maps = make_in_maps(student_features, teacher_features, codebook,
                           teacher_codes)
    res = bass_utils.run_bass_kernel_spmd(nc, in_maps,
                                          core_ids=list(range(NCORES)))
    return combine(res.results)

